# revision 10
# baseline (speedup 1.0000x reference)
"""Trainium2 Bass kernel v2 for grayscale+Canny+1x1-conv (nn_BFA_3015067042007).

Data-parallel over batch: 16 images -> 8 cores x 2 images.

v2 changes vs baseline (469us):
  - fp16 output DMA (host casts back to f32): halves the dominant out traffic.
  - conv restructured to 2 fp16 matmul banks (vs 3) via 16-row windows with
    K=128 = [xh(3c) | xl(3c) | ones | edge] slabs. xh/xl are a lossless host-
    side re-encoding of the f32 input (f16 hi + f16 residual), DMA'd directly
    in conv layout -> zero on-chip conversion ops. Bias rides the ones-slab.
  - edge slab distributed from the hysteresis result by one SBUF->SBUF DMA
    per strip (partition-crossing AP).
  - evictions widened to [128,1024] (2 PSUM banks) and rotated across
    ACT/DVE/Pool; out-DMAs widened to [128,4096] (512KB each).
  - two images software-pipelined: img1's P1-P5 interleaved with img0's conv
    so TensorE stays busy (avoids the cold-clock throttle) and engine phases
    overlap.

Bit-exactness: gray+floor+Sobel+NMS+hysteresis decisions replicate the
reference's f32 op order exactly (ops kept on the baseline's proven
engine/op/dtype choices; f16 tiles only hold exactly-representable integers).
"""

import numpy as np

B_FULL = 16
N_CORES = 8
B_LOC = B_FULL // N_CORES
H = 512
W_IMG = 512
NSTRIP = 4

MAGIC_A = 8388607.5
MAGIC_B = 8388608.0
TG22 = 0.4142135623730951
TG67 = 2.414213562373095

# shift-matrix stack indices
I_T_TOP, I_T_MID, I_T_BOT = 0, 1, 2
I_D_TOP, I_D_MID, I_D_BOT = 3, 4, 5
I_N, I_S, I_V = 6, 7, 8
I_H_TOP, I_H_BOT, I_H_TOP_D = 9, 10, 11
N_MATS = 12


def build_shift_mats():
    m = np.zeros((N_MATS, 128, 128), np.float16)
    i = np.arange(128)
    for t in (I_T_TOP, I_T_MID, I_T_BOT):
        m[t][i, i] = 2.0
        m[t][i[:-1], i[1:]] = 1.0
        m[t][i[1:], i[:-1]] = 1.0
    m[I_T_TOP][0, 0] = 3.0
    m[I_T_BOT][127, 127] = 3.0
    for t in (I_D_TOP, I_D_MID, I_D_BOT):
        m[t][i[1:], i[:-1]] = 1.0
        m[t][i[:-1], i[1:]] = -1.0
    m[I_D_TOP][0, 0] = -1.0
    m[I_D_BOT][127, 127] = 1.0
    m[I_N][i[:-1], i[1:]] = 1.0
    m[I_S][i[1:], i[:-1]] = 1.0
    m[I_V][i, i] = 1.0
    m[I_V][i[:-1], i[1:]] = 1.0
    m[I_V][i[1:], i[:-1]] = 1.0
    m[I_H_TOP][127, 0] = 1.0
    m[I_H_BOT][0, 127] = 1.0
    m[I_H_TOP_D][127, 0] = -1.0
    return m


def build_conv_banks(W, b):
    """Two fp16 lhsT banks per och-group for the K=128 conv.

    rhs tile partitions: [0:48]=xh (16c+r), [48:96]=xl, [96:112]=ones,
    [112:128]=edge (hysteresis 0/1). psum m = 8r + oi, och = 8g + oi.
      bankA: Wh on xh, bias on ones, Weh on edge
      bankB: Wl on xh, Wh on xl, Wel on edge
    where W = Wh + Wl (f16 split), We = 255*W[:,3] = Weh + Wel.
    Returns [4, 2, 128, 128] (g, bank, K, M)."""
    Wf = W.astype(np.float32)
    Wh = Wf[:, :3].astype(np.float16)
    Wl = (Wf[:, :3] - Wh.astype(np.float32)).astype(np.float16)
    We = (np.float32(255.0) * Wf[:, 3]).astype(np.float32)
    Weh = We.astype(np.float16)
    Wel = (We - Weh.astype(np.float32)).astype(np.float16)
    bh = b.astype(np.float16)
    banks = np.zeros((4, 2, 128, 128), np.float16)
    r = np.arange(16)
    for g in range(4):
        for oi in range(8):
            o = 8 * g + oi
            m = 8 * r + oi
            for c in range(3):
                banks[g, 0, 16 * c + r, m] = Wh[o, c]
                banks[g, 1, 16 * c + r, m] = Wl[o, c]
                banks[g, 1, 48 + 16 * c + r, m] = Wh[o, c]
            banks[g, 0, 96 + r, m] = bh[o]
            banks[g, 0, 112 + r, m] = Weh[o]
            banks[g, 1, 112 + r, m] = Wel[o]
    return banks


def pack_xi(x):
    """Host-side lossless re-encoding of x into conv rhs layout.

    Returns [B, NSTRIP, 112, 4096] f16: partition 16c+r = xh, 48+16c+r = xl,
    96..112 = ones; free col = 512*w + n (w = 16-row window in strip)."""
    B = x.shape[0]
    xf = np.ascontiguousarray(x, dtype=np.float32)
    xh = xf.astype(np.float16)
    xl = (xf - xh.astype(np.float32)).astype(np.float16)
    # [B, 3, H, W] -> [B, s, c, r, w, n] -> [B, s, 48, 4096]
    def lay(a):
        a = a.reshape(B, 3, NSTRIP, 8, 16, W_IMG)      # b c s w r n
        a = a.transpose(0, 2, 1, 4, 3, 5)              # b s c r w n
        return a.reshape(B, NSTRIP, 48, 8 * W_IMG)
    out = np.empty((B, NSTRIP, 112, 8 * W_IMG), np.float16)
    out[:, :, 0:48] = lay(xh)
    out[:, :, 48:96] = lay(xl)
    out[:, :, 96:112] = 1.0
    return out


_PROG_CACHE = {}


def build_program():
    import concourse.bacc as bacc
    import concourse.tile as tile
    import concourse.mybir as mybir
    from concourse.mybir import AluOpType as op, ActivationFunctionType as act
    from contextlib import ExitStack

    f32 = mybir.dt.float32
    f16 = mybir.dt.float16
    u8 = mybir.dt.uint8

    nc = bacc.Bacc("TRN2", target_bir_lowering=False, debug=False)
    x_d = nc.dram_tensor("x", [B_LOC, 3, H, W_IMG], f32, kind="ExternalInput").ap()
    xi_d = nc.dram_tensor("xi", [B_LOC, NSTRIP, 112, 4096], f16,
                          kind="ExternalInput").ap()
    mats_d = nc.dram_tensor("mats", [N_MATS, 128, 128], f16, kind="ExternalInput").ap()
    cb_d = nc.dram_tensor("cb", [8, 128, 128], f16, kind="ExternalInput").ap()
    # raw eviction dumps; host transposes to [B, 32, H, W]:
    # out[bi, s, wp, 8r+oi, 1024g+512wi+n] = y[bi, 8g+oi, 128s+32wp+16wi+r, n]
    out_d = nc.dram_tensor("out", [B_LOC, NSTRIP, 4, 128, 4096], f16,
                           kind="ExternalOutput").ap()

    with tile.TileContext(nc) as tc:
        with ExitStack() as ctx:
            ep = ctx.enter_context
            constp = ep(tc.tile_pool(name="const", bufs=1))
            rgbp = ep(tc.tile_pool(name="rgb", bufs=3))
            tmpp = ep(tc.tile_pool(name="tmp", bufs=2))
            gpadp = ep(tc.tile_pool(name="gpad", bufs=5))
            tplp = ep(tc.tile_pool(name="tpl", bufs=5))
            spadp = ep(tc.tile_pool(name="spad", bufs=3))
            sobp = ep(tc.tile_pool(name="sob", bufs=3))
            mskp = ep(tc.tile_pool(name="msk", bufs=5))
            keepp = ep(tc.tile_pool(name="keep", bufs=3))
            magp = ep(tc.tile_pool(name="magpad", bufs=5))
            nspp = ep(tc.tile_pool(name="nsp", bufs=3))
            selp = ep(tc.tile_pool(name="sel", bufs=3))
            weakp = ep(tc.tile_pool(name="weak", bufs=5))
            curp = ep(tc.tile_pool(name="cur", bufs=9))
            curfp = ep(tc.tile_pool(name="curf", bufs=4))
            hsp = ep(tc.tile_pool(name="hs", bufs=6))
            xip = ep(tc.tile_pool(name="xi", bufs=6))
            ovp = ep(tc.tile_pool(name="ov", bufs=3))
            pvertp = ep(tc.tile_pool(name="pvert", bufs=2, space="PSUM"))
            pconvp = ep(tc.tile_pool(name="pconv", bufs=3, space="PSUM"))

            mats = constp.tile([128, N_MATS, 128], f16, tag="mats")
            nc.sync.dma_start(mats[:], mats_d.rearrange("m k n -> k m n"))
            convb = constp.tile([128, 8, 128], f16, tag="convb")
            nc.sync.dma_start(convb[:], cb_d.rearrange("j k n -> k j n"))

            def mat(idx):
                return mats[:, idx, :]

            # per-image state
            st = [dict() for _ in range(B_LOC)]

            def p1s(bi, s):
                s_ = st[bi]
                if s == 0:
                    s_["gpads"] = []
                if True:
                    r0 = 128 * s
                    rgb = rgbp.tile([128, 3, 512], f32, tag="rgb")
                    nc.sync.dma_start(
                        rgb[:], x_d[bi, :, r0:r0 + 128, :].rearrange("c r n -> r c n"))
                    tr, tg, tb = rgb[:, 0, :], rgb[:, 1, :], rgb[:, 2, :]
                    g1 = tmpp.tile([128, 512], f32, tag="ta")
                    nc.vector.tensor_scalar(g1[:], tr, 0.2989, None, op0=op.mult)
                    g2 = tmpp.tile([128, 512], f32, tag="tb2")
                    nc.scalar.activation(g2[:], tg, act.Copy, bias=0.0, scale=0.587)
                    g3 = tmpp.tile([128, 512], f32, tag="tg3")
                    nc.gpsimd.tensor_tensor(g3[:], g1[:], g2[:], op=op.add)
                    g4 = tmpp.tile([128, 512], f32, tag="tb2")
                    nc.scalar.activation(g4[:], tb, act.Copy, bias=0.0, scale=0.114)
                    gray = tmpp.tile([128, 512], f32, tag="gray")
                    nc.vector.tensor_tensor(gray[:], g3[:], g4[:], op=op.add)
                    y1 = tmpp.tile([128, 512], f32, tag="ta")
                    nc.vector.tensor_scalar(y1[:], gray[:], MAGIC_A, None, op0=op.add)
                    z1 = tmpp.tile([128, 512], f32, tag="tb2")
                    nc.scalar.activation(z1[:], y1[:], act.Copy, bias=-MAGIC_B, scale=1.0)
                    d1 = tmpp.tile([128, 512], f32, tag="td")
                    nc.gpsimd.tensor_tensor(d1[:], gray[:], z1[:], op=op.subtract)
                    gpad = gpadp.tile([128, 514], f16, tag="gpad")
                    nc.vector.scalar_tensor_tensor(
                        gpad[:, 1:513], d1[:], 1.0, z1[:], op0=op.is_ge, op1=op.add)
                    nc.scalar.copy(gpad[:, 0:1], gpad[:, 1:2])
                    nc.scalar.copy(gpad[:, 513:514], gpad[:, 512:513])
                    s_["gpads"].append(gpad)

            def xi_load(bi):
                s_ = st[bi]
                s_["xis"] = []
                for s in range(NSTRIP):
                    xi = xip.tile([128, 4096], f16, tag="xi")
                    nc.sync.dma_start(xi[0:112, :], xi_d[bi, s])
                    s_["xis"].append(xi)

            def p2s(bi, s):
                s_ = st[bi]
                if s == 0:
                    s_["tpls"] = []
                if True:
                    gp = s_["gpads"][s]
                    u1 = tmpp.tile([128, 512], f16, tag="tc")
                    nc.vector.scalar_tensor_tensor(
                        u1[:], gp[:, 1:513], 2.0, gp[:, 0:512], op0=op.mult, op1=op.add)
                    tpl = tplp.tile([128, 512], f16, tag="tpl")
                    nc.gpsimd.tensor_tensor(tpl[:], u1[:], gp[:, 2:514], op=op.add)
                    s_["tpls"].append(tpl)

            def p3s(bi, s):
                s_ = st[bi]
                gpads, tpls = s_["gpads"], s_["tpls"]
                if s == 0:
                    s_["magpads"], s_["horizs"], s_["verts"], s_["ssns"] = [], [], [], []
                if True:
                    gp = gpads[s]
                    ps = pvertp.tile([128, 512], f32, tag="pv")
                    tm = (I_T_TOP, I_T_MID, I_T_MID, I_T_BOT)[s]
                    nc.tensor.matmul(ps[:], mat(tm), gp[:, 1:513], start=True, stop=False)
                    if s > 0:
                        nc.tensor.matmul(ps[:], mat(I_H_TOP), gpads[s - 1][:, 1:513],
                                         start=False, stop=(s == 3))
                    if s < 3:
                        nc.tensor.matmul(ps[:], mat(I_H_BOT), gpads[s + 1][:, 1:513],
                                         start=False, stop=True)
                    spad = spadp.tile([128, 514], f16, tag="spad")
                    nc.scalar.copy(spad[:, 1:513], ps[:])
                    nc.scalar.copy(spad[:, 0:1], spad[:, 1:2])
                    nc.scalar.copy(spad[:, 513:514], spad[:, 512:513])
                    gx = sobp.tile([128, 512], f16, tag="gx")
                    nc.gpsimd.tensor_tensor(gx[:], spad[:, 2:514], spad[:, 0:512],
                                            op=op.subtract)
                    pg = pvertp.tile([128, 512], f32, tag="pv")
                    dm = (I_D_TOP, I_D_MID, I_D_MID, I_D_BOT)[s]
                    nc.tensor.matmul(pg[:], mat(dm), tpls[s][:], start=True, stop=False)
                    if s > 0:
                        nc.tensor.matmul(pg[:], mat(I_H_TOP_D), tpls[s - 1][:],
                                         start=False, stop=(s == 3))
                    if s < 3:
                        nc.tensor.matmul(pg[:], mat(I_H_BOT), tpls[s + 1][:],
                                         start=False, stop=True)
                    gy = sobp.tile([128, 512], f16, tag="gy")
                    nc.scalar.copy(gy[:], pg[:])
                    ax = sobp.tile([128, 512], f16, tag="ax")
                    nc.scalar.activation(ax[:], gx[:], act.Abs)
                    ay = sobp.tile([128, 512], f16, tag="ay")
                    nc.scalar.activation(ay[:], gy[:], act.Abs)
                    magpad = magp.tile([128, 514], f16, tag="magpad")
                    nc.gpsimd.tensor_tensor(magpad[:, 1:513], ax[:], ay[:], op=op.add)
                    nc.vector.memset(magpad[:, 0:514:513], 0.0)
                    hz = mskp.tile([128, 512], u8, tag="hz")
                    nc.vector.scalar_tensor_tensor(
                        hz[:], ax[:], TG22, ay[:], op0=op.mult, op1=op.is_ge)
                    vt = mskp.tile([128, 512], u8, tag="vt")
                    nc.vector.scalar_tensor_tensor(
                        vt[:], ax[:], TG67, ay[:], op0=op.mult, op1=op.is_lt)
                    sprod = tmpp.tile([128, 512], f32, tag="sprod")
                    nc.vector.tensor_tensor(sprod[:], gx[:], gy[:], op=op.mult)
                    sn = mskp.tile([128, 512], u8, tag="sn")
                    nc.vector.tensor_scalar(sn[:], sprod[:], 0.0, None, op0=op.is_ge)
                    s_["magpads"].append(magpad)
                    s_["horizs"].append(hz)
                    s_["verts"].append(vt)
                    s_["ssns"].append(sn)

            def p4s(bi, s):
                s_ = st[bi]
                magpads = s_["magpads"]
                if s == 0:
                    s_["cur"] = []
                    s_["weaks"] = []
                if True:
                    mg = magpads[s]
                    pn = pvertp.tile([128, 512], f32, tag="pv")
                    nc.tensor.matmul(pn[:], mat(I_N), mg[:, 1:513],
                                     start=True, stop=(s == 0))
                    if s > 0:
                        nc.tensor.matmul(pn[:], mat(I_H_TOP), magpads[s - 1][:, 1:513],
                                         start=False, stop=True)
                    npad = nspp.tile([128, 514], f16, tag="npad")
                    nc.scalar.copy(npad[:, 1:513], pn[:])
                    nc.vector.memset(npad[:, 0:514:513], 0.0)
                    psS = pvertp.tile([128, 512], f32, tag="pv")
                    nc.tensor.matmul(psS[:], mat(I_S), mg[:, 1:513],
                                     start=True, stop=(s == 3))
                    if s < 3:
                        nc.tensor.matmul(psS[:], mat(I_H_BOT), magpads[s + 1][:, 1:513],
                                         start=False, stop=True)
                    spdS = nspp.tile([128, 514], f16, tag="spdS")
                    nc.scalar.copy(spdS[:, 1:513], psS[:])
                    nc.vector.memset(spdS[:, 0:514:513], 0.0)
                    fwd = selp.tile([128, 512], f16, tag="fwd")
                    nc.vector.tensor_copy(fwd[:], npad[:, 2:514])                       # ne
                    nc.vector.copy_predicated(fwd[:], s_["ssns"][s][:], npad[:, 0:512])  # nw
                    nc.vector.copy_predicated(fwd[:], s_["verts"][s][:], npad[:, 1:513])  # n
                    nc.vector.copy_predicated(fwd[:], s_["horizs"][s][:], mg[:, 2:514])  # e
                    bwd = selp.tile([128, 512], f16, tag="bwd")
                    nc.vector.tensor_copy(bwd[:], spdS[:, 0:512])                       # sw
                    nc.vector.copy_predicated(bwd[:], s_["ssns"][s][:], spdS[:, 2:514])  # se
                    nc.vector.copy_predicated(bwd[:], s_["verts"][s][:], spdS[:, 1:513])  # s
                    nc.vector.copy_predicated(bwd[:], s_["horizs"][s][:], mg[:, 0:512])  # w
                    bigm = selp.tile([128, 512], f16, tag="bigm")
                    nc.vector.scalar_tensor_tensor(
                        bigm[:], fwd[:], 1.0, bwd[:], op0=op.add, op1=op.max)
                    keep = keepp.tile([128, 512], f16, tag="keep")
                    nc.vector.tensor_tensor(keep[:], mg[:, 1:513], bigm[:], op=op.is_ge)
                    cpad = curp.tile([128, 514], f16, tag="cpad")
                    nc.vector.scalar_tensor_tensor(
                        cpad[:, 1:513], mg[:, 1:513], 150.0, keep[:],
                        op0=op.is_gt, op1=op.mult)
                    nc.vector.memset(cpad[:, 0:514:513], 0.0)
                    wk = weakp.tile([128, 512], f16, tag="wk")
                    nc.vector.scalar_tensor_tensor(
                        wk[:], mg[:, 1:513], 50.0, keep[:], op0=op.is_gt, op1=op.mult)
                    s_["cur"].append(cpad)
                    s_["weaks"].append(wk)

            def p5(bi):
                s_ = st[bi]
                cur = s_["cur"]
                for it in range(3):
                    last = (it == 2)
                    hts = [None] * NSTRIP
                    nxt = [None] * NSTRIP

                    def hstage(s):
                        cp = cur[s]
                        h1 = tmpp.tile([128, 512], f16, tag="tc")
                        nc.vector.tensor_tensor(h1[:], cp[:, 0:512], cp[:, 2:514],
                                                op=op.add)
                        ht = hsp.tile([128, 512], f16, tag="ht")
                        nc.vector.tensor_tensor(ht[:], h1[:], cp[:, 1:513], op=op.add)
                        hts[s] = ht

                    def vstage(s):
                        pv = pvertp.tile([128, 512], f32, tag="pv")
                        nc.tensor.matmul(pv[:], mat(I_V), hts[s][:], start=True,
                                         stop=False)
                        if s > 0:
                            nc.tensor.matmul(pv[:], mat(I_H_TOP), hts[s - 1][:],
                                             start=False, stop=(s == 3))
                        if s < 3:
                            nc.tensor.matmul(pv[:], mat(I_H_BOT), hts[s + 1][:],
                                             start=False, stop=True)
                        tsg = tmpp.tile([128, 512], f16, tag="tsg")
                        nc.scalar.activation(tsg[:], pv[:], act.Sign)
                        if last:
                            cf = curfp.tile([128, 512], f16, tag="cf")
                            nc.vector.tensor_tensor(cf[:], tsg[:], s_["weaks"][s][:],
                                                    op=op.mult)
                            nxt[s] = cf
                            xi = s_["xis"][s]
                            for w in range(8):
                                eng = nc.gpsimd if w % 2 == 0 else nc.sync
                                eng.dma_start(
                                    xi[112:128, 512 * w:512 * w + 512],
                                    cf[16 * w:16 * w + 16, :])
                        else:
                            cnew = curp.tile([128, 514], f16, tag="cpad")
                            nc.vector.tensor_tensor(cnew[:, 1:513], tsg[:],
                                                    s_["weaks"][s][:], op=op.mult)
                            nc.vector.memset(cnew[:, 0:514:513], 0.0)
                            nxt[s] = cnew

                    for s in range(NSTRIP):
                        hstage(s)
                    for s in range(NSTRIP):
                        vstage(s)
                    cur = nxt

            EV_ROTS = {0: ("act", "act", "act", "act", "act", "act", "act", "dve"),
                       1: ("act", "dve")}

            def p6_strip(bi, s, evctr=[0]):
                s_ = st[bi]
                xi = s_["xis"][s]
                for wp in range(4):  # window pairs: windows 2wp, 2wp+1
                    ov = ovp.tile([128, 4096], f16, tag="ov")
                    for g in range(4):
                        pc = pconvp.tile([128, 1024], f32, tag="pc")
                        for bank in range(2):
                            for wi in range(2):
                                w = 2 * wp + wi
                                nc.tensor.matmul(
                                    pc[:, 512 * wi:512 * wi + 512],
                                    convb[:, 2 * g + bank, :],
                                    xi[:, 512 * w:512 * w + 512],
                                    start=(bank == 0), stop=(bank == 1))
                        dst = ov[:, 1024 * g:1024 * (g + 1)]
                        rot = EV_ROTS[bi]
                        eng = rot[evctr[0] % len(rot)]
                        evctr[0] += 1
                        if eng == "act":
                            nc.scalar.activation(dst, pc[:], act.Relu,
                                                 bias=0.0, scale=1.0)
                        else:
                            nc.vector.tensor_scalar(dst, pc[:], 0.0, None, op0=op.max)
                    nc.sync.dma_start(out_d[bi, s, wp], ov[:])

            # ---- schedule ----
            def p1(bi):
                for s in range(NSTRIP):
                    p1s(bi, s)

            def p2(bi):
                for s in range(NSTRIP):
                    p2s(bi, s)

            def p3(bi):
                for s in range(NSTRIP):
                    p3s(bi, s)

            def p4(bi):
                for s in range(NSTRIP):
                    p4s(bi, s)

            p1(0); p2(0); xi_load(0); p3(0); p4(0); p5(0)
            p1(1)
            p6_strip(0, 0)
            p2(1)
            p6_strip(0, 1)
            xi_load(1)
            p3(1)
            p6_strip(0, 2)
            p4(1)
            p6_strip(0, 3)
            p5(1)
            for s in range(NSTRIP):
                p6_strip(1, s)

    nc.compile()
    return nc


def _get_program():
    if "nc" not in _PROG_CACHE:
        _PROG_CACHE["nc"] = build_program()
    return _PROG_CACHE["nc"]


def build_in_maps(x, W, b):
    x = np.ascontiguousarray(np.asarray(x, dtype=np.float32))
    W = np.asarray(W, dtype=np.float32)
    b = np.asarray(b, dtype=np.float32)
    mats = build_shift_mats()
    banks = build_conv_banks(W, b).reshape(8, 128, 128)
    xi = pack_xi(x)
    in_maps = []
    for core in range(N_CORES):
        sl = slice(B_LOC * core, B_LOC * (core + 1))
        in_maps.append({
            "x": np.ascontiguousarray(x[sl]),
            "xi": np.ascontiguousarray(xi[sl]),
            "mats": mats,
            "cb": np.ascontiguousarray(banks),
        })
    return in_maps


def unpack_out(raw):
    """[B, s, wp, 128, 4096] raw eviction dumps -> [B, 32, H, W] f32."""
    B = raw.shape[0]
    a = raw.reshape(B, 4, 4, 16, 8, 4, 2, 512)       # b s wp r oi g wi n
    a = a.transpose(0, 5, 4, 1, 2, 6, 3, 7)          # b g oi s wp wi r n
    return np.ascontiguousarray(a).reshape(B, 32, H, W_IMG).astype(np.float32)


def kernel(x: np.ndarray, W: np.ndarray, b: np.ndarray) -> np.ndarray:
    from concourse.bass_utils import run_bass_kernel_spmd

    nc = _get_program()
    in_maps = build_in_maps(x, W, b)
    res = run_bass_kernel_spmd(nc, in_maps, core_ids=list(range(N_CORES)))
    return np.concatenate([unpack_out(r["out"]) for r in res.results], axis=0)

